# revision 5
# baseline (speedup 1.0000x reference)
"""Cumulative link (ordinal) loss on 8 Trainium2 NeuronCores.

loss = mean_i [ -ln( sigmoid(hi_i - x_i) - sigmoid(lo_i - x_i) + eps ) ]
with per-label thresholds hi = [0,1,2,3,+inf][l], lo = [-inf,0,1,2,3][l].

Branch-free device formulation (l = label as float, G = l - x):
    S1 = sigmoid(G)          # = sigmoid(hi - x) when l <= 3
    S2 = sigmoid(G - 1)      # = sigmoid(lo - x) when l >= 1
    A  = max(S1, l - 3)      # l==4  ->  1,  else S1
    B  = min(S2, l)          # l==0  ->  0,  else S2
    p  = A - B
    partial = sum_free ln(p + eps)       (ACT Ln with accum_out)
Host: loss = -sum(partials) / B.

Perf notes:
  * All DVE elementwise ops run in fp16 so the 2x_1P perf mode engages
    (both operands 16-bit, dense).  fp16 keeps the numerics safe
    (verified ~9e-6 rel err end to end; bf16 S-values would be 1e-3).
  * Labels are DMAd densely as int32 pairs (the int64 words); the idle
    GpSimd engine converts the low words to dense fp16.
  * Logits are cast f32->fp16 during the (SWDGE) DMA itself.
  * In-place buffer chains: A overwrites G's slice, B overwrites S1's,
    P overwrites S2's, and the final Ln runs in place over P with a
    per-chunk accumulator.  Ln is forced after every sigmoid in ACT
    program order, so exactly one activation-table switch happens.

Sharding: pure data parallel, 1/8 of the batch per core, laid out
[128 partitions x 8192 free].
"""

import numpy as np

B_TOTAL = 8388608
N_CORES = 8
P = 128
SHARD = B_TOTAL // N_CORES          # 1048576 per core
M = SHARD // P                      # 8192 free-dim columns per core
T = 2048                            # tile width (columns)
NT = M // T
H = M // 2                          # Ln chunk width
EPS = 1e-8

_NC = None


def _build_nc():
    import concourse.bacc as bacc
    import concourse.mybir as mybir
    from concourse import tile
    from concourse.tile_rust import add_dep_helper

    f32 = mybir.dt.float32
    f16 = mybir.dt.float16
    i32 = mybir.dt.int32
    Alu = mybir.AluOpType
    Act = mybir.ActivationFunctionType

    nc = bacc.Bacc("TRN2", target_bir_lowering=False, debug=False)

    x_dram = nc.dram_tensor("logits", (P, M), f32, kind="ExternalInput")
    l_dram = nc.dram_tensor("labels", (P, 2 * M), i32, kind="ExternalInput")
    o_dram = nc.dram_tensor("out", (P, 2), f32, kind="ExternalOutput")

    def ts(t, w=T):
        return slice(t * w, (t + 1) * w)

    with tile.TileContext(nc) as tc:
        with tc.tile_pool(name="io", bufs=2) as iop, \
             tc.tile_pool(name="persist", bufs=1) as pp:
            bias_m1 = pp.tile([P, 1], f32, tag="bias_m1")
            nc.vector.memset(bias_m1[:], -1.0)
            bias_eps = pp.tile([P, 1], f32, tag="bias_eps")
            nc.vector.memset(bias_eps[:], EPS)

            lev_full = pp.tile([P, M], f16, tag="lev_full")
            g_full = pp.tile([P, M], f16, tag="g_full")    # G, then A
            s1_full = pp.tile([P, M], f16, tag="s1_full")  # S1, then B
            s2_full = pp.tile([P, M], f16, tag="s2_full")  # S2, then P, then ln
            acc = pp.tile([P, 2], f32, tag="acc")

            sigs = []
            lns = []
            for t in range(NT):
                x16 = iop.tile([P, T], f16, tag="x16")
                l32 = iop.tile([P, T, 2], i32, tag="l32")
                # cast DMA f32 -> fp16 (SWDGE); labels dense int32 (HWDGE)
                nc.gpsimd.dma_start(out=x16[:], in_=x_dram[:, ts(t)])
                nc.sync.dma_start(out=l32[:], in_=l_dram[:, ts(t, 2 * T)])
                lev = lev_full[:, ts(t)]
                # int32 low words (stride 2) -> dense fp16 on idle GpSimd
                nc.gpsimd.tensor_copy(out=lev, in_=l32[:, :, 0])

                g = g_full[:, ts(t)]
                s1 = s1_full[:, ts(t)]
                s2 = s2_full[:, ts(t)]
                nc.vector.tensor_tensor(out=g, in0=lev, in1=x16[:], op=Alu.subtract)
                sigs.append(nc.scalar.activation(s1, g, Act.Sigmoid))
                sigs.append(
                    nc.scalar.activation(s2, g, Act.Sigmoid, bias=bias_m1[:])
                )
                # A = max(l - 3, S1) -> overwrites G slice
                nc.vector.scalar_tensor_tensor(
                    out=g, in0=lev, scalar=3.0, in1=s1,
                    op0=Alu.subtract, op1=Alu.max,
                )
                # B = min(l, S2) -> overwrites S1 slice
                nc.vector.tensor_tensor(out=s1, in0=lev, in1=s2, op=Alu.min)
                # P = A - B -> overwrites S2 slice
                nc.vector.tensor_tensor(out=s2, in0=g, in1=s1, op=Alu.subtract)

            # ln(P + eps) in place, one accumulator column per half
            for h in range(2):
                lns.append(
                    nc.scalar.activation(
                        s2_full[:, ts(h, H)], s2_full[:, ts(h, H)], Act.Ln,
                        bias=bias_eps[:], accum_out=acc[:, h:h + 1],
                    )
                )
            # force every sigmoid before every Ln in ACT program order so the
            # activation table switches exactly once (sigmoid set -> ln set)
            for ln in lns:
                for sg in sigs:
                    add_dep_helper(
                        ln.ins, sg.ins, sync=False, reason="one act-table switch"
                    )
            nc.sync.dma_start(out=o_dram[:], in_=acc[:])

    nc.compile()
    return nc


def get_nc():
    global _NC
    if _NC is None:
        _NC = _build_nc()
    return _NC


def make_in_maps(logits, labels):
    x = np.ascontiguousarray(np.asarray(logits, dtype=np.float32)).reshape(B_TOTAL)
    lab = np.asarray(labels)
    if lab.dtype != np.int64:
        lab = lab.astype(np.int64)
    lab = np.ascontiguousarray(lab).reshape(B_TOTAL)
    lab32 = lab.view(np.int32)          # (2*B,) interleaved low/high words
    in_maps = []
    for c in range(N_CORES):
        xs = x[c * SHARD:(c + 1) * SHARD].reshape(P, M)
        ls = lab32[c * 2 * SHARD:(c + 1) * 2 * SHARD].reshape(P, 2 * M)
        in_maps.append({"logits": xs, "labels": ls})
    return in_maps


def run(logits, labels, trace=False):
    """Returns (loss_scalar_f32, BassKernelResults)."""
    from concourse.bass_utils import run_bass_kernel_spmd

    nc = get_nc()
    in_maps = make_in_maps(logits, labels)
    res = run_bass_kernel_spmd(
        nc, in_maps, core_ids=list(range(N_CORES)), trace=trace
    )
    total = 0.0
    for r in res.results:
        total += r["out"].astype(np.float64).sum()
    loss = np.float32(-total / B_TOTAL)
    return np.asarray(loss), res


def kernel(logits, labels):
    out, _ = run(logits, labels, trace=False)
    return out
